# revision 3
# baseline (speedup 1.0000x reference)
"""KD loss (teacher softmax x student log-softmax, masked mean) on 8 TRN2 cores.

Sharding: data-parallel over the 4096 tokens -- 512 tokens per core.
Each core streams its (512, 32000) slices of student/teacher logits once
and emits per-(token, vocab-chunk) partial sums; the host finishes the
tiny remaining reduction in float64.

Per token t over vocab i:
    Z_t  = sum_i exp(teacher_i)
    Z_x  = sum_i exp(student_i)
    cross = sum_i exp(teacher_i) * student_i
    x_t  = cross / Z_t - ln(Z_x)           # = sum_i p_i * logsoftmax(x)_i
    loss = -sum_t x_t * mask_t / sum_t mask_t

No max-subtraction: inputs are standard normal (|logit| < ~6), so exp is
safe in fp32 and sums (~5e4) are well within range.

v3 layout: profiling shows SDMA engine 15 runs ~17% slower than engines
0-14 on this part, and the per-partition engine swizzle assigns it
partitions {92-95, 124-127}.  Main tiles therefore use partitions
[0:126) -- engine 15 carries 6 of 126 lines (~0.76x of the even share),
matching its speed -- and the 8 leftover tokens per core are spread as
16 vocab-sixteenths across all 128 partitions in one small tail tile.

The host interleaves teacher/student chunkwise so each (tile, window) is
ONE HWDGE DMA with 64 KB contiguous lines.  exp(teacher) runs in place
over the left half of the tile; the io pool triple-buffers whole
windows; all DMAs stay on the sync ring so completions are in order.
"""

import numpy as np

_B, _S, _V = 2, 2048, 32000
_N = _B * _S                      # 4096 tokens
_NCORES = 8
_TOK = _N // _NCORES              # 512 tokens per core
_PM = 126                         # partitions per main tile (engine 15 diet)
_NTILES = 4                       # main tiles per core
_TMAIN = _PM * _NTILES            # 504 main tokens per core
_TMINI = _TOK - _TMAIN            # 8 tail tokens per core
_F = 8000                         # vocab chunk per window
_NW = _V // _F                    # 4 windows per tile
_FMINI = _V // 16                 # 2000: sixteenth-block vocab per partition
_NCOLS = _NTILES * _NW + 1        # 17 stat columns per statistic

_cache = {}


def _build():
    import concourse.bacc as bacc
    import concourse.mybir as mybir
    import concourse.tile as tile

    f32 = mybir.dt.float32
    AF = mybir.ActivationFunctionType
    ALU = mybir.AluOpType

    nc = bacc.Bacc()
    # interleaved rows: [T_c0 | S_c0 | T_c1 | S_c1 | ...] per token
    main = nc.dram_tensor("main", [_TMAIN, 2 * _V], f32, kind="ExternalInput")
    # tail tokens as sixteenth-blocks: row p = [T_q | S_q] of token p//16
    mini = nc.dram_tensor("mini", [128, 2 * _FMINI], f32, kind="ExternalInput")
    # raw per-chunk stats, host finishes: cols [0:17]=Z_t, [17:34]=Z_x,
    # [34:51]=cross
    out = nc.dram_tensor("out", [128, 3 * _NCOLS], f32, kind="ExternalOutput")

    with tile.TileContext(nc) as tc:
        with (
            tc.tile_pool(name="io", bufs=3) as io,
            tc.tile_pool(name="sink", bufs=2) as sink,
            tc.tile_pool(name="stats", bufs=1) as stats,
        ):
            stats_all = stats.tile([128, 3 * _NCOLS], f32)

            def cols(base, k):
                return stats_all[:, base * _NCOLS + k : base * _NCOLS + k + 1]

            def chunk_ops(t2, p, fch, k):
                """exp/accumulate ops for one loaded window of p partitions."""
                tT = t2[:, :fch]
                tX = t2[:, fch : 2 * fch]
                zt = cols(0, k)[:p]
                zx = cols(1, k)[:p]
                cr = cols(2, k)[:p]
                # exp(teacher) in place, fused free-dim accum -> Z_t
                nc.scalar.activation(tT, tT, AF.Exp, accum_out=zt)
                # exp(student): only its free-dim sum is needed, so the
                # full output is discarded through a stride-0 AP
                xsink = sink.tile([p, 1], f32)
                nc.scalar.activation(
                    xsink.broadcast_to((p, fch)), tX, AF.Exp, accum_out=zx
                )
                # cross partial: one fused DVE multiply+accumulate
                psink = sink.tile([p, 1], f32)
                nc.vector.scalar_tensor_tensor(
                    out=psink.broadcast_to((p, fch)),
                    in0=tT,
                    scalar=1.0,
                    in1=tX,
                    op0=ALU.mult,
                    op1=ALU.mult,
                    accum_out=cr,
                )

            for it in range(_NTILES):
                rows = slice(it * _PM, (it + 1) * _PM)
                for j in range(_NW):
                    csl = slice(2 * j * _F, 2 * (j + 1) * _F)
                    t2 = io.tile([_PM, 2 * _F], f32)
                    nc.sync.dma_start(out=t2[:, :], in_=main[rows, csl])
                    chunk_ops(t2, _PM, _F, it * _NW + j)

            # tail: 8 tokens x 16 sixteenths on all 128 partitions
            t2 = io.tile([128, 2 * _FMINI], f32)
            nc.sync.dma_start(out=t2[:, :], in_=mini[:, :])
            chunk_ops(t2, 128, _FMINI, _NCOLS - 1)

            nc.sync.dma_start(out=out[:, :], in_=stats_all[:, :])

    nc.finalize()
    return nc


def _interleave(student_2d, teacher_2d):
    """Per-core DRAM images: main [8, 504, 64000], mini [8, 128, 4000]."""
    xs_m = np.empty((_NCORES, _TMAIN, 2 * _V), dtype=np.float32)
    xs_t = np.empty((_NCORES, 128, 2 * _FMINI), dtype=np.float32)
    t3 = teacher_2d.reshape(_NCORES, _TOK, _NW, _F)
    s3 = student_2d.reshape(_NCORES, _TOK, _NW, _F)
    # main: [T_c | S_c] interleave at F granularity
    m = np.stack([t3[:, :_TMAIN], s3[:, :_TMAIN]], axis=3)
    xs_m[:] = m.reshape(_NCORES, _TMAIN, 2 * _V)
    # mini: token 504+i, sixteenth q -> partition i*16+q
    tt = teacher_2d.reshape(_NCORES, _TOK, 16, _FMINI)[:, _TMAIN:]
    ss = student_2d.reshape(_NCORES, _TOK, 16, _FMINI)[:, _TMAIN:]
    mm = np.stack([tt, ss], axis=3)           # [8, 8, 16, 2, 2000]
    xs_t[:] = mm.reshape(_NCORES, 128, 2 * _FMINI)
    return xs_m, xs_t


def _run(student_2d, teacher_2d, trace=False):
    """student_2d/teacher_2d: (4096, 32000) f32 C-contiguous.
    Returns (x_tokens[4096] float64, BassKernelResults)."""
    from concourse.bass_utils import run_bass_kernel_spmd

    if "nc" not in _cache:
        _cache["nc"] = _build()
    nc = _cache["nc"]

    xs_m, xs_t = _interleave(student_2d, teacher_2d)

    in_maps = []
    for c in range(_NCORES):
        in_maps.append(
            {
                "main": np.ascontiguousarray(xs_m[c]),
                "mini": np.ascontiguousarray(xs_t[c]),
            }
        )
    res = run_bass_kernel_spmd(
        nc, in_maps, core_ids=list(range(_NCORES)), trace=trace
    )
    raw = np.stack([r["out"] for r in res.results])  # [8, 128, 51]

    xt = np.empty(_N, dtype=np.float64)
    for c in range(_NCORES):
        st = raw[c].astype(np.float64)
        zt = np.zeros(_TOK)
        zx = np.zeros(_TOK)
        cr = np.zeros(_TOK)
        for it in range(_NTILES):
            ks = [it * _NW + j for j in range(_NW)]
            rows = slice(it * _PM, (it + 1) * _PM)
            zt[rows] = st[:_PM, ks].sum(axis=1)
            zx[rows] = st[:_PM, [_NCOLS + k for k in ks]].sum(axis=1)
            cr[rows] = st[:_PM, [2 * _NCOLS + k for k in ks]].sum(axis=1)
        k = _NCOLS - 1
        zt[_TMAIN:] = st[:, k].reshape(_TMINI, 16).sum(axis=1)
        zx[_TMAIN:] = st[:, _NCOLS + k].reshape(_TMINI, 16).sum(axis=1)
        cr[_TMAIN:] = st[:, 2 * _NCOLS + k].reshape(_TMINI, 16).sum(axis=1)
        xt[c * _TOK : (c + 1) * _TOK] = cr / zt - np.log(zx)
    return xt, res


def kernel(logits, teacher_logits, labels):
    lg = np.ascontiguousarray(np.asarray(logits, dtype=np.float32).reshape(_N, _V))
    tg = np.ascontiguousarray(
        np.asarray(teacher_logits, dtype=np.float32).reshape(_N, _V)
    )
    xt, _ = _run(lg, tg, trace=False)
    lab = np.asarray(labels).reshape(_N)
    mask = lab != -100
    loss = -(xt[mask].sum()) / max(int(mask.sum()), 1)
    return np.asarray(loss, dtype=np.float32)


# revision 4
# speedup vs baseline: 1.4645x; 1.4645x over previous
"""KD loss (teacher softmax x student log-softmax, masked mean) on 8 TRN2 cores.

Sharding: data-parallel over the 4096 tokens -- 512 tokens per core.
Each core streams its (512, 32000) slices of student/teacher logits once
and emits per-(token, vocab-chunk) partial sums; the host finishes the
tiny remaining reduction in float64.

Per token t over vocab i:
    Z_t  = sum_i exp(teacher_i)
    Z_x  = sum_i exp(student_i)
    cross = sum_i exp(teacher_i) * student_i
    x_t  = cross / Z_t - ln(Z_x)           # = sum_i p_i * logsoftmax(x)_i
    loss = -sum_t x_t * mask_t / sum_t mask_t

No max-subtraction: inputs are standard normal (|logit| < ~6), so exp is
safe in fp32 and sums (~5e4) are well within range.

v4 port-balanced layout.  Profiling shows SBUF port/engine 15 (serving
partitions 120-127) sustains only ~21.9 GB/s vs ~27.1 GB/s for ports
0-14, and that DMA lines are dealt to engines by line index in equal
blocks (a 128-line DMA -> 16 engines x 8 lines, port-affine; a 120-line
DMA -> 15 engines x 8 lines, engine 15 idle; anything else fragments).
So every DMA is either 128 or 120 lines:

  - per tile (128 tokens), [0:128] windows cover vocab [0, 26180)
    (widths 8000+8000+8000+2180),
  - a [0:120] window gives low tokens their last 5820 vocab,
  - high tokens' (partitions 120-127) last 5820 vocab is shed to one
    [0:120] window per core as 15 blocks of 388 vocab per token
    (480 blocks = 120 rows x 4 slots).

Port 15 then moves 6.7 MB vs 8.29 MB for ports 0-14 -- both ~306 us.
Teacher/student stay chunk-interleaved so each window is ONE DMA;
exp(teacher) runs in place; io pool triple-buffers; all DMAs on the
sync ring.
"""

import numpy as np

_B, _S, _V = 2, 2048, 32000
_N = _B * _S                      # 4096 tokens
_NCORES = 8
_TOK = _N // _NCORES              # 512 tokens per core
_P = 128                          # partitions (tokens per tile)
_PLO = 120                        # low partitions (ports 0-14)
_NTILES = _TOK // _P              # 4 tiles per core
_WF = [8000, 8000, 8000, 2180]    # full-window vocab widths, sum 26180
_VF = sum(_WF)                    # 26180: vocab covered on all 128 rows
_VLO = _V - _VF                   # 5820: vocab in the [0:120] windows
_U = 388                          # shed block vocab (5820 = 15 * 388)
_NSLOT = 4                        # shed slots per row (480 blocks/120 rows)
_NHI = _NTILES * (_P - _PLO)      # 32 high tokens per core
# stat columns: per tile 4 full + 1 lo-own, then 4 shed slots
_CPT = len(_WF) + 1
_NCOLS = _NTILES * _CPT + _NSLOT  # 24

_cache = {}


def _build():
    import concourse.bacc as bacc
    import concourse.mybir as mybir
    import concourse.tile as tile

    f32 = mybir.dt.float32
    AF = mybir.ActivationFunctionType
    ALU = mybir.AluOpType

    nc = bacc.Bacc()
    # per-token row: [T|S] interleaved per window; rows >= 120 of each
    # tile pad the last 2*_VLO floats (never read by any DMA)
    main = nc.dram_tensor("main", [_TOK, 2 * _V], f32, kind="ExternalInput")
    # shed blocks: row r slot b = [T_388|S_388] of high token (r*4+b)//15
    shed = nc.dram_tensor(
        "shed", [_PLO, 2 * _U * _NSLOT], f32, kind="ExternalInput"
    )
    # raw stats: cols [0:24]=Z_t, [24:48]=Z_x, [48:72]=cross
    out = nc.dram_tensor("out", [_P, 3 * _NCOLS], f32, kind="ExternalOutput")

    with tile.TileContext(nc) as tc:
        with (
            tc.tile_pool(name="io", bufs=3) as io,
            tc.tile_pool(name="sink", bufs=2) as sink,
            tc.tile_pool(name="stats", bufs=1) as stats,
        ):
            stats_all = stats.tile([_P, 3 * _NCOLS], f32)

            def col(base, k, p):
                return stats_all[:p, base * _NCOLS + k : base * _NCOLS + k + 1]

            def chunk_ops(tT, tX, p, fch, k):
                """exp/accumulate ops for one [p, fch] T/S slice pair."""
                # exp(teacher) in place, fused free-dim accum -> Z_t
                nc.scalar.activation(tT, tT, AF.Exp, accum_out=col(0, k, p))
                # exp(student): only its free-dim sum is needed; the full
                # output is discarded through a stride-0 AP
                xsink = sink.tile([p, 1], f32)
                nc.scalar.activation(
                    xsink.broadcast_to((p, fch)), tX, AF.Exp,
                    accum_out=col(1, k, p),
                )
                # cross partial: one fused DVE multiply+accumulate
                psink = sink.tile([p, 1], f32)
                nc.vector.scalar_tensor_tensor(
                    out=psink.broadcast_to((p, fch)),
                    in0=tT,
                    scalar=1.0,
                    in1=tX,
                    op0=ALU.mult,
                    op1=ALU.mult,
                    accum_out=col(2, k, p),
                )

            for it in range(_NTILES):
                r0 = it * _P
                off = 0
                for j, w in enumerate(_WF):
                    t2 = io.tile([_P, 2 * w], f32)
                    nc.sync.dma_start(
                        out=t2[:, :],
                        in_=main[r0 : r0 + _P, 2 * off : 2 * (off + w)],
                    )
                    chunk_ops(t2[:, :w], t2[:, w : 2 * w], _P, w, it * _CPT + j)
                    off += w
                # low tokens' remaining vocab on partitions [0:120)
                t2 = io.tile([_PLO, 2 * _VLO], f32)
                nc.sync.dma_start(
                    out=t2[:, :],
                    in_=main[r0 : r0 + _PLO, 2 * _VF : 2 * _V],
                )
                chunk_ops(
                    t2[:, :_VLO], t2[:, _VLO : 2 * _VLO], _PLO, _VLO,
                    it * _CPT + len(_WF),
                )

            # shed window: high tokens' remaining vocab as 388-blocks
            t2 = io.tile([_PLO, 2 * _U * _NSLOT], f32)
            nc.sync.dma_start(out=t2[:, :], in_=shed[:, :])
            for b in range(_NSLOT):
                o = 2 * _U * b
                chunk_ops(
                    t2[:, o : o + _U], t2[:, o + _U : o + 2 * _U], _PLO, _U,
                    _NTILES * _CPT + b,
                )

            nc.sync.dma_start(out=out[:, :], in_=stats_all[:, :])

    nc.finalize()
    return nc


def _wf_offsets():
    offs, o = [], 0
    for w in _WF:
        offs.append(o)
        o += w
    return offs


def _interleave(student_2d, teacher_2d):
    """Per-core DRAM images: main [8, 512, 64000], shed [8, 120, 3104]."""
    t = teacher_2d.reshape(_NCORES, _TOK, _V)
    s = student_2d.reshape(_NCORES, _TOK, _V)
    xs_m = np.empty((_NCORES, _TOK, 2 * _V), dtype=np.float32)
    o2 = 0
    for w, o in zip(_WF, _wf_offsets()):
        xs_m[:, :, o2 : o2 + w] = t[:, :, o : o + w]
        xs_m[:, :, o2 + w : o2 + 2 * w] = s[:, :, o + w - w : o + w]
        o2 += 2 * w
    # [0:120] windows: low rows' tail vocab (high rows: padding, unread)
    xs_m[:, :, 2 * _VF : 2 * _VF + _VLO] = t[:, :, _VF:]
    xs_m[:, :, 2 * _VF + _VLO :] = s[:, :, _VF:]

    # shed: block g = (hi_token_idx*15 + slice); row g//4, slot g%4
    xs_h = np.empty((_NCORES, _PLO, 2 * _U * _NSLOT), dtype=np.float32)
    hi_rows = (
        np.arange(_NTILES)[:, None] * _P + (_PLO + np.arange(_P - _PLO))[None, :]
    ).reshape(-1)                                  # 32 per core
    th = t[:, hi_rows, _VF:].reshape(_NCORES, _NHI * 15, _U)  # [8,480,388]
    sh = s[:, hi_rows, _VF:].reshape(_NCORES, _NHI * 15, _U)
    blk = np.stack([th, sh], axis=2)               # [8, 480, 2, 388]
    xs_h[:] = blk.reshape(_NCORES, _PLO, _NSLOT, 2 * _U).reshape(
        _NCORES, _PLO, 2 * _U * _NSLOT
    )
    return xs_m, xs_h


def _run(student_2d, teacher_2d, trace=False):
    """student_2d/teacher_2d: (4096, 32000) f32 C-contiguous.
    Returns (x_tokens[4096] float64, BassKernelResults)."""
    from concourse.bass_utils import run_bass_kernel_spmd

    if "nc" not in _cache:
        _cache["nc"] = _build()
    nc = _cache["nc"]

    xs_m, xs_h = _interleave(student_2d, teacher_2d)

    in_maps = []
    for c in range(_NCORES):
        in_maps.append(
            {
                "main": np.ascontiguousarray(xs_m[c]),
                "shed": np.ascontiguousarray(xs_h[c]),
            }
        )
    res = run_bass_kernel_spmd(
        nc, in_maps, core_ids=list(range(_NCORES)), trace=trace
    )
    raw = np.stack([r["out"] for r in res.results])  # [8, 128, 72]

    xt = np.empty(_N, dtype=np.float64)
    for c in range(_NCORES):
        st = raw[c].astype(np.float64)
        zt = np.zeros((_NTILES, _P))
        zx = np.zeros((_NTILES, _P))
        cr = np.zeros((_NTILES, _P))
        for it in range(_NTILES):
            # full windows: all 128 rows
            kf = [it * _CPT + j for j in range(len(_WF))]
            zt[it] = st[:, kf].sum(axis=1)
            zx[it] = st[:, [_NCOLS + k for k in kf]].sum(axis=1)
            cr[it] = st[:, [2 * _NCOLS + k for k in kf]].sum(axis=1)
            # lo-own window: rows 0-119
            kl = it * _CPT + len(_WF)
            zt[it, :_PLO] += st[:_PLO, kl]
            zx[it, :_PLO] += st[:_PLO, _NCOLS + kl]
            cr[it, :_PLO] += st[:_PLO, 2 * _NCOLS + kl]
        # shed: high token h (0..31) has blocks g = h*15 .. h*15+14
        ks = [_NTILES * _CPT + b for b in range(_NSLOT)]
        shz = st[:_PLO, ks].reshape(-1)            # 480 in g order
        shx = st[:_PLO, [_NCOLS + k for k in ks]].reshape(-1)
        shc = st[:_PLO, [2 * _NCOLS + k for k in ks]].reshape(-1)
        for h in range(_NHI):
            it, p = h // (_P - _PLO), _PLO + h % (_P - _PLO)
            zt[it, p] += shz[h * 15 : (h + 1) * 15].sum()
            zx[it, p] += shx[h * 15 : (h + 1) * 15].sum()
            cr[it, p] += shc[h * 15 : (h + 1) * 15].sum()
        x = cr.reshape(-1) / zt.reshape(-1) - np.log(zx.reshape(-1))
        xt[c * _TOK : (c + 1) * _TOK] = x
    return xt, res


def kernel(logits, teacher_logits, labels):
    lg = np.ascontiguousarray(np.asarray(logits, dtype=np.float32).reshape(_N, _V))
    tg = np.ascontiguousarray(
        np.asarray(teacher_logits, dtype=np.float32).reshape(_N, _V)
    )
    xt, _ = _run(lg, tg, trace=False)
    lab = np.asarray(labels).reshape(_N)
    mask = lab != -100
    loss = -(xt[mask].sum()) / max(int(mask.sum()), 1)
    return np.asarray(loss, dtype=np.float32)
